# revision 1
# baseline (speedup 1.0000x reference)
"""DenseAtt GNN message-passing kernel for Trainium2 (8 NeuronCores).

Computes out = adj * sigmoid(s_left[:, None] + s_right[None, :] + b)
with s_left = x @ W[:F], s_right = x @ W[F:], for x [N, F], adj [N, N].

Sharding: 1D row partition of adj / out across the 8 cores (1024 rows each).

Per-core pipeline (paired column chunks, row blocks interleaved):
  - s_right broadcast: host stages x^T as float16; one PE matmul per
    512-column chunk with lhsT = w_right replicated across all 128 output
    partitions computes s_right[j] broadcast down every partition, straight
    into PSUM. No AllGather, no replication pass, no PSUM->SBUF copy.
  - s_left: DVE multiplies the core's x rows (natural layout) by the
    broadcast w_left and reduces over features -> per-row-block bias.
  - stream: two column chunks' srr tiles live in PSUM at once and row
    blocks alternate between them, so each sigmoid's scheduler-assigned
    completion-chain wait lands two ACT ops back and is long satisfied --
    the sigmoids run back-to-back instead of paying a ~220ns semaphore
    round-trip each. adj tiles arrive as float16 (host downcast halves the
    dominant read; ~4e-4 relative error against the 2e-2 gate), ACT applies
    sigmoid reading s_right from PSUM with the per-row-block bias, DVE (or
    GPSIMD for a deterministic subset, to balance engines) multiplies by
    adj in f32 into the two halves of a pair-wide att tile, and one SWDGE
    kv_writeback per (row block, chunk pair) returns the f32 result to HBM
    (batch=1 / d_head=128 / ncn=2*CCH, dho_stride = out row stride) --
    the stripe-wise descriptor pricing beats a DMACopy ~14x and the wide
    ncn halves the Pool desc-gen load per byte.
"""

import sys

import numpy as np

sys.path.insert(0, "/opt/trn_rl_repo")

N = 8192
F = 128
NCORES = 8
RPC = N // NCORES  # rows per core: 1024
P = 128
NBLK = RPC // P  # row blocks per core: 8
CCH = 2048  # streamed column chunk
NCCH = N // CCH

_nc = None
ADJ_BUFS = 8
ATT_BUFS = 8  # column-pair att tiles [P, 2*CCH]
XT_BUFS = 2
# row blocks whose att*adj mul runs on GPSIMD (engine balancing); none in
# the last column chunk so the drain stays on the faster DVE
POOL_MUL = {0: (1, 4), 1: (1, 5), 2: (1, 4), 3: (2, 5)}


def _build():
    from contextlib import ExitStack

    import concourse.tile as tile
    from concourse import bacc, mybir

    f32 = mybir.dt.float32
    f16 = mybir.dt.float16

    nc = bacc.Bacc(
        "TRN2",
        target_bir_lowering=False,
        debug=False,
        enable_asserts=True,
        num_devices=NCORES,
    )

    adj = nc.dram_tensor("adj", [RPC, N], f16, kind="ExternalInput").ap()
    xt = nc.dram_tensor("xt", [F, N], f16, kind="ExternalInput").ap()
    wrep = nc.dram_tensor("wrep", [F, P], f16, kind="ExternalInput").ap()
    xr = nc.dram_tensor("xr", [RPC, F], f32, kind="ExternalInput").ap()
    # packed head: x row-block 0 | w_left broadcast | bias, one DMA
    hd = nc.dram_tensor("hd", [P, 2 * F + 1], f32, kind="ExternalInput").ap()
    out = nc.dram_tensor("out", [RPC, N], f32, kind="ExternalOutput").ap()

    with tile.TileContext(nc) as tc, ExitStack() as ctx:
        const_pool = ctx.enter_context(tc.tile_pool(name="const", bufs=1))
        xbuf_pool = ctx.enter_context(tc.tile_pool(name="xbuf", bufs=1))
        xt_pool = ctx.enter_context(tc.tile_pool(name="xt", bufs=XT_BUFS))
        adj_pool = ctx.enter_context(tc.tile_pool(name="adj", bufs=ADJ_BUFS))
        att_pool = ctx.enter_context(tc.tile_pool(name="att", bufs=ATT_BUFS))
        srr_pool = ctx.enter_context(tc.tile_pool(name="srr", bufs=2, space="PSUM"))

        # DMA-queue head, ordered by what gates the first sigmoid: the first
        # x^T chunk (srr matmuls), then wrep, then the packed head
        # (x row-block 0 | w_left | b) for the row-0 bias, then the
        # remaining x rows
        xt0 = xt_pool.tile([F, CCH], f16, tag="xt")
        nc.sync.dma_start(xt0[:], xt[:, 0:CCH])
        wrep_sb = const_pool.tile([F, P], f16, tag="wrep")
        nc.sync.dma_start(wrep_sb[:], wrep)
        hd_sb = xbuf_pool.tile([P, 2 * F + 1], f32, tag="hd")
        nc.sync.dma_start(hd_sb[:], hd)
        xr0 = hd_sb[:, 0:F]
        wl_sb = hd_sb[:, F : 2 * F]
        b_sb = hd_sb[:, 2 * F : 2 * F + 1]
        # x rows 128..1023 in natural layout [p, c*F + f] = x_rows[c*P + p, f]
        xr_nat = xbuf_pool.tile([P, RPC - F], f32)
        nc.sync.dma_start(
            xr_nat[:].rearrange("p (c f) -> p c f", f=F),
            xr[P:RPC].rearrange("(c p) f -> p c f", p=P),
        )

        cst = const_pool.tile([P, 16], f32)
        sl_sb = cst[:, 4:12]  # s_left + b, row block rb in col rb
        zidx = const_pool.tile([P, 1], mybir.dt.int32, tag="zidx")
        nc.vector.memset(zidx, 0.0)
        # dummy early sigmoid: pulls the ACT table load off the critical path
        nc.vector.memset(cst[:, 2:3], 0.0)
        nc.scalar.activation(
            cst[:, 3:4],
            cst[:, 2:3],
            mybir.ActivationFunctionType.Sigmoid,
            bias=cst[:, 2:3],
        )

        # s_left: tmp = x_rows * w_left per feature, reduce over f, add bias.
        # Row block 0 runs standalone (from the packed head load) so the
        # first sigmoid's bias is ready early; blocks 1..7 batch afterwards.
        tmp = xbuf_pool.tile([P, RPC], f32, tag="tmp")
        s2l = const_pool.tile([P, NBLK], f32, tag="s2l")
        nc.vector.tensor_mul(tmp[:, 0:F], xr0, wl_sb)
        nc.vector.tensor_reduce(
            s2l[:, 0:1],
            tmp[:, 0:F][:].rearrange("p (c f) -> p c f", f=F),
            mybir.AxisListType.X,
            mybir.AluOpType.add,
        )
        nc.vector.tensor_scalar_add(sl_sb[:, 0:1], s2l[:, 0:1], b_sb)
        # blocks 1..7 one at a time so sl_c lands progressively, just ahead
        # of row block c's first sigmoid
        for c in range(1, NBLK):
            nc.vector.tensor_mul(
                tmp[:, c * F : (c + 1) * F],
                xr_nat[:, (c - 1) * F : c * F],
                wl_sb,
            )
            nc.vector.tensor_reduce(
                s2l[:, c : c + 1],
                tmp[:, c * F : (c + 1) * F][:].rearrange("p (c f) -> p c f", f=F),
                mybir.AxisListType.X,
                mybir.AluOpType.add,
            )
            nc.vector.tensor_scalar_add(sl_sb[:, c : c + 1], s2l[:, c : c + 1], b_sb)

        # out rows viewed as [row_block, dhi=128, dho=1, col] for
        # kv_writeback; the pair's two adjacent column chunks are contiguous,
        # so one ncn=2*CCH writeback covers both — halving the SWDGE
        # desc-gen load per written byte
        out4 = out.rearrange("(A r d) c -> A r d c", r=P, d=1)

        def write_att2(att2, rb, ccp):
            in4 = att2[:].rearrange("p (d b n) -> p d b n", d=1, b=1)
            cols = slice(2 * ccp * CCH, (2 * ccp + 2) * CCH)
            nc.gpsimd.kv_writeback(out4[rb : rb + 1, :, :, cols], in4, zidx[:])

        # paired column chunks, row blocks interleaved across the pair:
        # consecutive sigmoids read ALTERNATING srr PSUM tiles, so the tile
        # scheduler's same-engine completion chain lands two ops back (its
        # semaphore long since fired) instead of serializing each sigmoid
        # behind the previous one's ~220ns sem round-trip
        def build_srr(cc):
            if cc == 0:
                xt_t = xt0
            else:
                xt_t = xt_pool.tile([F, CCH], f16, tag="xt")
                nc.sync.dma_start(xt_t[:], xt[:, cc * CCH : (cc + 1) * CCH])
            srr = srr_pool.tile([P, CCH], f32, tag="srr")
            for i in range(CCH // 512):
                nc.tensor.matmul(
                    srr[:, i * 512 : (i + 1) * 512],
                    wrep_sb[:],
                    xt_t[:, i * 512 : (i + 1) * 512],
                )
            return srr

        for ccp in range(NCCH // 2):
            cca, ccb = 2 * ccp, 2 * ccp + 1
            srr_a = build_srr(cca)
            srr_b = build_srr(ccb)
            for rb in range(NBLK):
                att2 = att_pool.tile([P, 2 * CCH], f32, tag="att")
                last = ccp == NCCH // 2 - 1 and rb == NBLK - 1
                for half, (cc, srr) in enumerate(((cca, srr_a), (ccb, srr_b))):
                    cols = slice(cc * CCH, (cc + 1) * CCH)
                    hs = slice(half * CCH, (half + 1) * CCH)
                    adj_t = adj_pool.tile([P, CCH], f16, tag="adj")
                    nc.sync.dma_start(adj_t[:], adj[rb * P : (rb + 1) * P, cols])
                    nc.scalar.activation(
                        att2[:, hs],
                        srr[:],
                        mybir.ActivationFunctionType.Sigmoid,
                        bias=sl_sb[:, rb : rb + 1],
                    )
                    eng = nc.gpsimd if rb in POOL_MUL[cc] else nc.vector
                    if last and half == 1:
                        # final drain: 1024-col mul pieces + their own
                        # writebacks shorten the serial end-of-stream chain
                        for q in range(2):
                            qs = slice(half * CCH + q * 1024, half * CCH + (q + 1) * 1024)
                            nc.vector.tensor_mul(
                                att2[:, qs], att2[:, qs], adj_t[:, q * 1024 : (q + 1) * 1024]
                            )
                    else:
                        eng.tensor_mul(att2[:, hs], att2[:, hs], adj_t[:])
                if last:
                    in4a = att2[:, 0:CCH].rearrange("p (d b n) -> p d b n", d=1, b=1)
                    nc.gpsimd.kv_writeback(
                        out4[rb : rb + 1, :, :, slice(2 * ccp * CCH, (2 * ccp + 1) * CCH)],
                        in4a, zidx[:],
                    )
                    for q in range(2):
                        qcols = slice((2 * ccp + 1) * CCH + q * 1024, (2 * ccp + 1) * CCH + (q + 1) * 1024)
                        in4q = att2[:, CCH + q * 1024 : CCH + (q + 1) * 1024].rearrange(
                            "p (d b n) -> p d b n", d=1, b=1
                        )
                        nc.gpsimd.kv_writeback(out4[rb : rb + 1, :, :, qcols], in4q, zidx[:])
                else:
                    write_att2(att2, rb, ccp)

    nc.compile()
    return nc


def kernel(x, adj, W, b):
    global _nc
    x = np.ascontiguousarray(np.asarray(x, dtype=np.float32))
    adj = np.asarray(adj, dtype=np.float32)
    W = np.asarray(W, dtype=np.float32).reshape(2 * F)
    b = np.float32(np.asarray(b).reshape(()))

    if _nc is None:
        _nc = _build()

    xt_np = np.ascontiguousarray(x.T.astype(np.float16))
    wrep_np = np.ascontiguousarray(
        np.broadcast_to(W[F:, None].astype(np.float16), (F, P))
    )
    wl_np = np.broadcast_to(W[None, :F], (P, F))

    in_maps = []
    for k in range(NCORES):
        rows = slice(k * RPC, (k + 1) * RPC)
        hd_np = np.empty((P, 2 * F + 1), dtype=np.float32)
        hd_np[:, 0:F] = x[k * RPC : k * RPC + P]
        hd_np[:, F : 2 * F] = wl_np
        hd_np[:, 2 * F] = b
        im = {
            "adj": np.ascontiguousarray(adj[rows].astype(np.float16)),
            "xt": xt_np,
            "wrep": wrep_np,
            "xr": np.ascontiguousarray(x[rows]),
            "hd": hd_np,
        }
        in_maps.append(im)

    import time

    from concourse.bass_utils import run_bass_kernel_spmd

    res = None
    for attempt in range(4):
        try:
            res = run_bass_kernel_spmd(_nc, in_maps, core_ids=list(range(NCORES)))
            break
        except Exception:
            # transient device wedges clear after a short wait; retry
            if attempt == 3:
                raise
            time.sleep(40 * (attempt + 1))
    return np.concatenate([r["out"] for r in res.results], axis=0)



# revision 4
# speedup vs baseline: 1.1899x; 1.1899x over previous
"""DenseAtt GNN message-passing kernel for Trainium2 (8 NeuronCores).

Computes out = adj * sigmoid(s_left[:, None] + s_right[None, :] + b)
with s_left = x @ W[:F], s_right = x @ W[F:], for x [N, F], adj [N, N].

Sharding: 1D row partition of adj / out across the 8 cores (1024 rows each).

Per-core design (32 tiles of [128, 2048], graded by the Tile cost model):

  sigmoid(sl + sr) = 1 / (1 + u*v) with u = e^-(sl+b) (per-row scalar) and
  v = e^-sr (per-column vector), so a tile can be produced EITHER by the
  ACT engine's sigmoid table (input: s_right replicated in PSUM, per-row
  bias) or by a single custom DVE op

      out = adj8 * recip_1nr(v*u + 1)

  (BITWISE_NOT fp32 exponent-flip seed + one Newton step, ~1.7e-3 rel err)
  which folds denominator, reciprocal and the adj multiply into one 1x DVE
  pass. PSUM only holds two [128, 2048] f32 s_right-replicated tiles, so:

  - chunks 0-1 (cols 0..4095): fused DVE path; their srr tiles are consumed
    by ACT Exp into SBUF f16 v-tiles during the early Exp-table phase, then
    recycled. One DVE op per tile, no ACT, no Pool.
  - chunks 2-3 (cols 4096..8191): srr stays resident in PSUM; ACT sigmoid
    (bias = sl+b) writes f16 att in place, then the adj multiply runs on
    DVE at 2x (f16-staged adj, 7 tiles) or on Pool (u8 adj, 9 tiles) to
    balance DVE/Pool/DMA at ~43 us each.

  adj is staged u8 (round(255*adj), quant err ~2e-3 abs) except the 7
  DVE-mult tiles which are staged f16 (255*adj) so every operand of the 2x
  TensorTensor is 2-byte. All paths produce 255*adj*att in f16; one
  kv_writeback per row-block pair (batch=2, ncn=8192, stripe-descriptor
  pricing) returns them to HBM and the host upcasts + rescales by 1/255.

  s_left comes from 8 tiny PE matmuls against the core's own x^T slice;
  ACT Exp/Identity (every table set) turn it into u and the sigmoid bias,
  so the program needs exactly one Exp->Sigmoid table switch.
"""

import sys

import numpy as np

sys.path.insert(0, "/opt/trn_rl_repo")

N = 8192
F = 128
NCORES = 8
RPC = N // NCORES  # rows per core: 1024
P = 128
NBLK = RPC // P  # row blocks per core: 8
CCH = 2048
NCCH = N // CCH  # 4 column chunks

# chunks 0-1: fused DVE path. chunks 2-3: sigmoid; mult engine per (rb, cc):
POOL_MULT = {(0, 2), (2, 2), (4, 2), (6, 2), (0, 3), (1, 3), (3, 3), (5, 3), (7, 3)}

# 1-Newton reciprocal constants (equioscillating over the x*bitcast(~x)
# seed interval [-4.5, -4]; max rel err 1.73e-3)
RC0, RC1 = -0.23549792, 2.0017324

_nc = None
_FUSED = None


def _register_fused_op():
    """Register the custom DVE op  out = Src1 * recip_1nr(Src0*C0 + 1).

    C0 carries the per-partition u scalar; C1/C2 the reciprocal constants.
    The BITWISE_NOT seed operates on the internal fp32 value of z, so in0
    may be f16 and in1 u8.
    """
    global _FUSED
    if _FUSED is not None:
        return _FUSED
    import concourse.dve_ops as dve_ops
    from concourse.dve_spec import AluOp, Bin, C0, C1, C2, One, Spec, Src0, Src1, lower
    from concourse.dve_uop import DveOpSpec

    _z = Src0 * C0 + One
    _nz = Bin(AluOp.BITWISE_NOT, _z, _z)
    _w0 = _nz * C1
    _w1 = _w0 * (C2 - _z * _w0)

    def _ref(in0, in1, c0, c1, c2):
        z = (in0.astype(np.float32) * c0 + 1.0).astype(np.float32)
        nz = (~z.view(np.int32)).view(np.float32)
        w0 = nz * c1
        w1 = w0 * (c2 - z * w0)
        return in1.astype(np.float32) * w1

    spec = Spec(body=Src1 * _w1, reference=_ref)
    name = "FUSED_SIG_MUL"
    row = 17
    shas = {}
    for ver in ("v3", "v4"):
        uops = lower(spec, ver=ver)
        shas[ver] = DveOpSpec(name=name, opcode=row, uops=uops, rd1_en=True).sha(ver)
    op = dve_ops.DveOp(name, spec, subdim=False, uops_sha=shas)
    if not any(o.name == name for o in dve_ops.OPS):
        dve_ops.OPS.append(op)
    dve_ops.CUSTOM_DVE_SPECS[name] = spec
    dve_ops._SUB_OPCODE_FOR_NAME[name] = row
    _FUSED = op
    return op


def _build():
    from contextlib import ExitStack

    import concourse.tile as tile
    from concourse import bacc, mybir

    fused_op = _register_fused_op()

    f32 = mybir.dt.float32
    f16 = mybir.dt.float16
    u8 = mybir.dt.uint8

    nc = bacc.Bacc(
        "TRN2",
        target_bir_lowering=False,
        debug=False,
        enable_asserts=True,
        num_devices=NCORES,
    )

    adj8 = nc.dram_tensor("adj8", [RPC, N], u8, kind="ExternalInput").ap()
    adj16 = nc.dram_tensor("adj16", [RPC, N], f16, kind="ExternalInput").ap()
    xt = nc.dram_tensor("xt", [F, N], f16, kind="ExternalInput").ap()
    xlt = nc.dram_tensor("xlt", [F, RPC], f16, kind="ExternalInput").ap()
    wl = nc.dram_tensor("wl", [F, 1], f16, kind="ExternalInput").ap()
    wrep = nc.dram_tensor("wrep", [F, P], f16, kind="ExternalInput").ap()
    bvec = nc.dram_tensor("bvec", [P, 2], f32, kind="ExternalInput").ap()
    out = nc.dram_tensor("out", [RPC, N], f16, kind="ExternalOutput").ap()

    Sig = mybir.ActivationFunctionType.Sigmoid
    Exp = mybir.ActivationFunctionType.Exp
    Ident = mybir.ActivationFunctionType.Identity

    with tile.TileContext(nc) as tc, ExitStack() as ctx:
        const_pool = ctx.enter_context(tc.tile_pool(name="const", bufs=1))
        xt_pool = ctx.enter_context(tc.tile_pool(name="xt", bufs=2))
        v_pool = ctx.enter_context(tc.tile_pool(name="v", bufs=2))
        adj8_pool = ctx.enter_context(tc.tile_pool(name="adj8", bufs=6))
        adj16_pool = ctx.enter_context(tc.tile_pool(name="adj16", bufs=4))
        att_pool = ctx.enter_context(tc.tile_pool(name="att", bufs=2))
        ps_pool = ctx.enter_context(tc.tile_pool(name="ps", bufs=2, space="PSUM"))

        # DMA order = DMA-engine service order: xt0 gates v0 (the first fused
        # tiles), xlt/wl gate u and the sigmoid biases, then xt1..3.
        xt_t = [None] * NCCH
        xt_t[0] = xt_pool.tile([F, CCH], f16, tag="xt", name="xt_sb")
        nc.sync.dma_start(xt_t[0][:], xt[:, 0:CCH])
        xlt_sb = const_pool.tile([F, RPC], f16)
        nc.sync.dma_start(xlt_sb[:], xlt)
        wl_sb = const_pool.tile([F, 1], f16)
        nc.sync.dma_start(wl_sb[:], wl)
        wrep_sb = const_pool.tile([F, P], f16)
        nc.sync.dma_start(wrep_sb[:], wrep)
        bvec_sb = const_pool.tile([P, 2], f32)
        nc.sync.dma_start(bvec_sb[:], bvec)
        for cc in range(1, NCCH):
            xt_t[cc] = xt_pool.tile([F, CCH], f16, tag="xt", name="xt_sb")
            nc.sync.dma_start(xt_t[cc][:], xt[:, cc * CCH : (cc + 1) * CCH])

        zidx = const_pool.tile([P, 2], mybir.dt.int32)
        nc.vector.memset(zidx, 0.0)

        # s_left via PE into the corner of the first PSUM buffer; buffer is
        # recycled for srr1 once u/slb have read it.
        slps = ps_pool.tile([P, CCH], f32, tag="ps")
        for c in range(NBLK):
            nc.tensor.matmul(
                slps[:, c : c + 1], xlt_sb[:, c * P : (c + 1) * P], wl_sb[:]
            )
        srr = [None] * NCCH
        for cc in range(NCCH):
            srr[cc] = ps_pool.tile([P, CCH], f32, tag="ps", name="srr")
            for i in range(CCH // 512):
                nc.tensor.matmul(
                    srr[cc][:, i * 512 : (i + 1) * 512],
                    wrep_sb[:],
                    xt_t[cc][:, i * 512 : (i + 1) * 512],
                )

        # Exp-table phase: u = e^-(sl+b), slb = sl+b, v_cc = e^-srr (cc 0,1)
        u_sb = const_pool.tile([P, NBLK], f32)
        nc.scalar.activation(u_sb[:], slps[:, 0:NBLK], Exp, scale=-1.0,
                             bias=bvec_sb[:, 0:1])
        slb_sb = const_pool.tile([P, NBLK], f32)
        nc.scalar.activation(slb_sb[:], slps[:, 0:NBLK], Ident,
                             bias=bvec_sb[:, 1:2])
        v_t = [None, None]
        for cc in range(2):
            v_t[cc] = v_pool.tile([P, CCH], f16, tag="v", name="v_sb")
            nc.scalar.activation(v_t[cc][:], srr[cc][:], Exp, scale=-1.0)

        out4 = out.rearrange("(A r d) c -> A r d c", r=P, d=1)

        for rbp in range(NBLK // 2):
            att2 = att_pool.tile([P, 2 * N], f16, tag="att")
            for half in range(2):
                rb = 2 * rbp + half
                for cc in range(NCCH):
                    seg = att2[:, half * N + cc * CCH : half * N + (cc + 1) * CCH]
                    cols = slice(cc * CCH, (cc + 1) * CCH)
                    if cc < 2:
                        a8 = adj8_pool.tile([P, CCH], u8, tag="a8")
                        nc.sync.dma_start(a8[:], adj8[rb * P : (rb + 1) * P, cols])
                        nc.vector._custom_dve(
                            fused_op, out=seg, in0=v_t[cc][:], in1=a8[:],
                            s0=u_sb[:, rb : rb + 1], s1=RC0, imm2=RC1,
                        )
                    else:
                        nc.scalar.activation(seg, srr[cc][:], Sig,
                                             bias=slb_sb[:, rb : rb + 1])
                        if (rb, cc) in POOL_MULT:
                            a8 = adj8_pool.tile([P, CCH], u8, tag="a8")
                            nc.sync.dma_start(a8[:], adj8[rb * P : (rb + 1) * P, cols])
                            nc.gpsimd.tensor_mul(seg, seg, a8[:])
                        else:
                            a16 = adj16_pool.tile([P, CCH], f16, tag="a16")
                            nc.sync.dma_start(a16[:], adj16[rb * P : (rb + 1) * P, cols])
                            nc.vector.tensor_mul(seg, seg, a16[:])
            in4 = att2[:].rearrange("p (d b n) -> p d b n", d=1, b=2)
            nc.gpsimd.kv_writeback(out4[2 * rbp : 2 * rbp + 2, :, :, :], in4, zidx[:])

    nc.compile()
    return nc


def kernel(x, adj, W, b):
    global _nc
    x = np.ascontiguousarray(np.asarray(x, dtype=np.float32))
    adj = np.asarray(adj, dtype=np.float32)
    W = np.asarray(W, dtype=np.float32).reshape(2 * F)
    b = np.float32(np.asarray(b).reshape(()))

    if _nc is None:
        _nc = _build()

    xt_np = np.ascontiguousarray(x.T.astype(np.float16))
    wl_np = np.ascontiguousarray(W[:F, None].astype(np.float16))
    wrep_np = np.ascontiguousarray(
        np.broadcast_to(W[F:, None].astype(np.float16), (F, P))
    )
    bvec_np = np.stack([np.full(P, -b), np.full(P, b)], axis=1).astype(np.float32)

    in_maps = []
    for k in range(NCORES):
        rows = slice(k * RPC, (k + 1) * RPC)
        adj_rows = adj[rows]
        im = {
            "adj8": np.ascontiguousarray(np.rint(adj_rows * 255.0).astype(np.uint8)),
            "adj16": np.ascontiguousarray((adj_rows * 255.0).astype(np.float16)),
            "xt": xt_np,
            "xlt": np.ascontiguousarray(x[rows].T.astype(np.float16)),
            "wl": wl_np,
            "wrep": wrep_np,
            "bvec": bvec_np,
        }
        in_maps.append(im)

    import time

    from concourse.bass_utils import run_bass_kernel_spmd

    res = None
    for attempt in range(4):
        try:
            res = run_bass_kernel_spmd(_nc, in_maps, core_ids=list(range(NCORES)))
            break
        except Exception:
            # transient device wedges clear after a short wait; retry
            if attempt == 3:
                raise
            time.sleep(40 * (attempt + 1))
    scale = np.float32(1.0 / 255.0)
    return np.concatenate(
        [np.asarray(r["out"], dtype=np.float32) * scale for r in res.results], axis=0
    )


# revision 8
# speedup vs baseline: 1.2012x; 1.0095x over previous
"""DenseAtt GNN message-passing kernel for Trainium2 (8 NeuronCores).

Computes out = adj * sigmoid(s_left[:, None] + s_right[None, :] + b)
with s_left = x @ W[:F], s_right = x @ W[F:], for x [N, F], adj [N, N].

Sharding: 1D row partition of adj / out across the 8 cores (1024 rows each).

Per-core design (32 tiles of [128, 2048], graded by the Tile cost model):

  sigmoid(sl + sr) = 1 / (1 + u*v) with u = e^-(sl+b) (per-row scalar) and
  v = e^-sr (per-column vector), so a tile can be produced EITHER by the
  ACT engine's sigmoid table (input: s_right replicated in PSUM, per-row
  bias) or by a single custom DVE op

      out = adj8 * recip_1nr(v*u + 1)

  (BITWISE_NOT fp32 exponent-flip seed + one Newton step, ~1.7e-3 rel err)
  which folds denominator, reciprocal and the adj multiply into one 1x DVE
  pass. PSUM only holds two [128, 2048] f32 s_right-replicated tiles, so:

  - chunks 0-1 (cols 0..4095): fused DVE path; their srr tiles are consumed
    by ACT Exp into SBUF f16 v-tiles during the early Exp-table phase, then
    recycled. One DVE op per tile, no ACT, no Pool.
  - chunks 2-3 (cols 4096..8191): srr stays resident in PSUM; ACT sigmoid
    (bias = sl+b) writes f16 att in place, then the adj multiply runs on
    DVE at 2x (f16-staged adj, 7 tiles) or on Pool (u8 adj, 9 tiles) to
    balance DVE/Pool/DMA at ~43 us each.

  adj is staged u8 (round(255*adj), quant err ~2e-3 abs) except the 7
  DVE-mult tiles which are staged f16 (255*adj) so every operand of the 2x
  TensorTensor is 2-byte. All paths produce 255*adj*att in f16; one
  kv_writeback per row-block pair (batch=2, ncn=8192, stripe-descriptor
  pricing) returns them to HBM and the host upcasts + rescales by 1/255.

  s_left comes from 8 tiny PE matmuls against the core's own x^T slice;
  ACT Exp/Identity (every table set) turn it into u and the sigmoid bias,
  so the program needs exactly one Exp->Sigmoid table switch.
"""

import sys

import numpy as np

sys.path.insert(0, "/opt/trn_rl_repo")

N = 8192
F = 128
NCORES = 8
RPC = N // NCORES  # rows per core: 1024
P = 128
NBLK = RPC // P  # row blocks per core: 8
CCH = 2048
NCCH = N // CCH  # 4 column chunks

# chunks 0-1: fused DVE path. chunks 2-3: sigmoid; mult engine per (rb, cc):
POOL_MULT = {(0, 2), (2, 2), (4, 2), (6, 2), (1, 3), (3, 3), (5, 3)}

# 1-Newton reciprocal constants (equioscillating over the x*bitcast(~x)
# seed interval [-4.5, -4]; max rel err 1.73e-3)
RC0, RC1 = -0.23549792, 2.0017324

_nc = None
_FUSED = None


def _register_fused_op():
    """Register the custom DVE op  out = Src1 * recip_1nr(Src0*C0 + 1).

    C0 carries the per-partition u scalar; C1/C2 the reciprocal constants.
    The BITWISE_NOT seed operates on the internal fp32 value of z, so in0
    may be f16 and in1 u8.
    """
    global _FUSED
    if _FUSED is not None:
        return _FUSED
    import concourse.dve_ops as dve_ops
    from concourse.dve_spec import AluOp, Bin, C0, C1, C2, One, Spec, Src0, Src1, lower
    from concourse.dve_uop import DveOpSpec

    _z = Src0 * C0 + One
    _nz = Bin(AluOp.BITWISE_NOT, _z, _z)
    _w0 = _nz * C1
    _w1 = _w0 * (C2 - _z * _w0)

    def _ref(in0, in1, c0, c1, c2):
        z = (in0.astype(np.float32) * c0 + 1.0).astype(np.float32)
        nz = (~z.view(np.int32)).view(np.float32)
        w0 = nz * c1
        w1 = w0 * (c2 - z * w0)
        return in1.astype(np.float32) * w1

    spec = Spec(body=Src1 * _w1, reference=_ref)
    name = "FUSED_SIG_MUL"
    row = 17
    shas = {}
    for ver in ("v3", "v4"):
        uops = lower(spec, ver=ver)
        shas[ver] = DveOpSpec(name=name, opcode=row, uops=uops, rd1_en=True).sha(ver)
    op = dve_ops.DveOp(name, spec, subdim=False, uops_sha=shas)
    if not any(o.name == name for o in dve_ops.OPS):
        dve_ops.OPS.append(op)
    dve_ops.CUSTOM_DVE_SPECS[name] = spec
    dve_ops._SUB_OPCODE_FOR_NAME[name] = row
    _FUSED = op
    return op


def _build():
    from contextlib import ExitStack

    import concourse.tile as tile
    from concourse import bacc, mybir

    fused_op = _register_fused_op()

    f32 = mybir.dt.float32
    f16 = mybir.dt.float16
    u8 = mybir.dt.uint8

    nc = bacc.Bacc(
        "TRN2",
        target_bir_lowering=False,
        debug=False,
        enable_asserts=True,
        num_devices=NCORES,
    )

    adj8 = nc.dram_tensor("adj8", [RPC, N], u8, kind="ExternalInput").ap()
    adj16 = nc.dram_tensor("adj16", [RPC, N], f16, kind="ExternalInput").ap()
    xt = nc.dram_tensor("xt", [F, N], f16, kind="ExternalInput").ap()
    xlt = nc.dram_tensor("xlt", [F, RPC], f16, kind="ExternalInput").ap()
    wl = nc.dram_tensor("wl", [F, 1], f16, kind="ExternalInput").ap()
    wrep = nc.dram_tensor("wrep", [F, P], f16, kind="ExternalInput").ap()
    bvec = nc.dram_tensor("bvec", [P, 2], f32, kind="ExternalInput").ap()
    out = nc.dram_tensor("out", [RPC, N], f16, kind="ExternalOutput").ap()

    Sig = mybir.ActivationFunctionType.Sigmoid
    Exp = mybir.ActivationFunctionType.Exp
    Ident = mybir.ActivationFunctionType.Identity

    with tile.TileContext(nc) as tc, ExitStack() as ctx:
        const_pool = ctx.enter_context(tc.tile_pool(name="const", bufs=1))
        xt_pool = ctx.enter_context(tc.tile_pool(name="xt", bufs=2))
        v_pool = ctx.enter_context(tc.tile_pool(name="v", bufs=2))
        adj8_pool = ctx.enter_context(tc.tile_pool(name="adj8", bufs=8))
        adj16_pool = ctx.enter_context(tc.tile_pool(name="adj16", bufs=6))
        att_pool = ctx.enter_context(tc.tile_pool(name="att", bufs=3))
        ps_pool = ctx.enter_context(tc.tile_pool(name="ps", bufs=2, space="PSUM"))

        # adj tile loader: tiles are issued in an explicit early-prefetch
        # order (interleaved with the xt loads below) so the first fused op
        # is gated by v0, not by its adj DMA sitting behind 9us of loads.
        adj_tiles = {}

        def load_adj(rb, cc):
            cols = slice(cc * CCH, (cc + 1) * CCH)
            if cc >= 2 and (rb, cc) not in POOL_MULT:
                t = adj16_pool.tile([P, CCH], f16, tag="a16", name="a16")
                nc.sync.dma_start(t[:], adj16[rb * P : (rb + 1) * P, cols])
            else:
                t = adj8_pool.tile([P, CCH], u8, tag="a8", name="a8")
                nc.sync.dma_start(t[:], adj8[rb * P : (rb + 1) * P, cols])
            adj_tiles[(rb, cc)] = t

        # DMA order = DMA-engine service order: xt0+wrep gate srr0 -> v0 (the
        # first fused tiles), xlt/wl gate u and the sigmoid biases; rb0's adj
        # tiles slot between the remaining xt chunks.
        xt_t = [None] * NCCH
        xt_t[0] = xt_pool.tile([F, CCH], f16, tag="xt", name="xt_sb")
        nc.sync.dma_start(xt_t[0][:], xt[:, 0:CCH])
        wrep_sb = const_pool.tile([F, P], f16)
        nc.sync.dma_start(wrep_sb[:], wrep)
        xlt_sb = const_pool.tile([F, RPC], f16)
        nc.sync.dma_start(xlt_sb[:], xlt)
        wl_sb = const_pool.tile([F, 1], f16)
        nc.sync.dma_start(wl_sb[:], wl)
        load_adj(0, 0)
        xt_t[1] = xt_pool.tile([F, CCH], f16, tag="xt", name="xt_sb")
        nc.sync.dma_start(xt_t[1][:], xt[:, CCH : 2 * CCH])
        load_adj(0, 1)
        xt_t[2] = xt_pool.tile([F, CCH], f16, tag="xt", name="xt_sb")
        nc.sync.dma_start(xt_t[2][:], xt[:, 2 * CCH : 3 * CCH])
        load_adj(1, 0)
        xt_t[3] = xt_pool.tile([F, CCH], f16, tag="xt", name="xt_sb")
        nc.sync.dma_start(xt_t[3][:], xt[:, 3 * CCH : 4 * CCH])
        bvec_sb = const_pool.tile([P, 2], f32)
        nc.sync.dma_start(bvec_sb[:], bvec)
        load_adj(0, 2)
        load_adj(1, 1)
        load_adj(0, 3)
        for rb in range(1, NBLK):
            for cc in range(NCCH):
                if (rb, cc) not in adj_tiles:
                    load_adj(rb, cc)

        zidx = const_pool.tile([P, 2], mybir.dt.int32)
        nc.vector.memset(zidx, 0.0)

        # s_left via PE into the corner of the first PSUM buffer; buffer is
        # recycled for srr1 once u/slb have read it.
        slps = ps_pool.tile([P, CCH], f32, tag="ps")
        for c in range(NBLK):
            nc.tensor.matmul(
                slps[:, c : c + 1], xlt_sb[:, c * P : (c + 1) * P], wl_sb[:]
            )
        srr = [None] * NCCH
        for cc in range(NCCH):
            srr[cc] = ps_pool.tile([P, CCH], f32, tag="ps", name="srr")
            for i in range(CCH // 512):
                nc.tensor.matmul(
                    srr[cc][:, i * 512 : (i + 1) * 512],
                    wrep_sb[:],
                    xt_t[cc][:, i * 512 : (i + 1) * 512],
                )

        # Exp-table phase: u = e^-(sl+b), slb = sl+b, v_cc = e^-srr (cc 0,1)
        u_sb = const_pool.tile([P, NBLK], f32)
        nc.scalar.activation(u_sb[:], slps[:, 0:NBLK], Exp, scale=-1.0,
                             bias=bvec_sb[:, 0:1])
        slb_sb = const_pool.tile([P, NBLK], f32)
        nc.scalar.activation(slb_sb[:], slps[:, 0:NBLK], Ident,
                             bias=bvec_sb[:, 1:2])
        v_t = [None, None]
        for cc in range(2):
            v_t[cc] = v_pool.tile([P, CCH], f16, tag="v", name="v_sb")
            nc.scalar.activation(v_t[cc][:], srr[cc][:], Exp, scale=-1.0)

        out4 = out.rearrange("(A r d) c -> A r d c", r=P, d=1)

        for rbp in range(NBLK // 2):
            att2 = att_pool.tile([P, 2 * N], f16, tag="att")
            for half in range(2):
                rb = 2 * rbp + half
                for cc in range(NCCH):
                    seg = att2[:, half * N + cc * CCH : half * N + (cc + 1) * CCH]
                    a_t = adj_tiles[(rb, cc)]
                    if cc < 2:
                        nc.vector._custom_dve(
                            fused_op, out=seg, in0=v_t[cc][:], in1=a_t[:],
                            s0=u_sb[:, rb : rb + 1], s1=RC0, imm2=RC1,
                        )
                    else:
                        nc.scalar.activation(seg, srr[cc][:], Sig,
                                             bias=slb_sb[:, rb : rb + 1])
                        if (rb, cc) in POOL_MULT:
                            nc.gpsimd.tensor_mul(seg, seg, a_t[:])
                        else:
                            nc.vector.tensor_mul(seg, seg, a_t[:])
            in4 = att2[:].rearrange("p (d b n) -> p d b n", d=1, b=2)
            nc.gpsimd.kv_writeback(out4[2 * rbp : 2 * rbp + 2, :, :, :], in4, zidx[:])

    nc.compile()
    return nc


def kernel(x, adj, W, b):
    global _nc
    x = np.ascontiguousarray(np.asarray(x, dtype=np.float32))
    adj = np.asarray(adj, dtype=np.float32)
    W = np.asarray(W, dtype=np.float32).reshape(2 * F)
    b = np.float32(np.asarray(b).reshape(()))

    if _nc is None:
        _nc = _build()

    xt_np = np.ascontiguousarray(x.T.astype(np.float16))
    wl_np = np.ascontiguousarray(W[:F, None].astype(np.float16))
    wrep_np = np.ascontiguousarray(
        np.broadcast_to(W[F:, None].astype(np.float16), (F, P))
    )
    bvec_np = np.stack([np.full(P, -b), np.full(P, b)], axis=1).astype(np.float32)

    in_maps = []
    for k in range(NCORES):
        rows = slice(k * RPC, (k + 1) * RPC)
        adj_rows = adj[rows]
        im = {
            "adj8": np.ascontiguousarray(np.rint(adj_rows * 255.0).astype(np.uint8)),
            "adj16": np.ascontiguousarray((adj_rows * 255.0).astype(np.float16)),
            "xt": xt_np,
            "xlt": np.ascontiguousarray(x[rows].T.astype(np.float16)),
            "wl": wl_np,
            "wrep": wrep_np,
            "bvec": bvec_np,
        }
        in_maps.append(im)

    import time

    from concourse.bass_utils import run_bass_kernel_spmd

    res = None
    for attempt in range(4):
        try:
            res = run_bass_kernel_spmd(_nc, in_maps, core_ids=list(range(NCORES)))
            break
        except Exception:
            # transient device wedges clear after a short wait; retry
            if attempt == 3:
                raise
            time.sleep(40 * (attempt + 1))
    scale = np.float32(1.0 / 255.0)
    return np.concatenate(
        [np.asarray(r["out"], dtype=np.float32) * scale for r in res.results], axis=0
    )
